# revision 1
# baseline (speedup 1.0000x reference)
"""Bass/Trainium2 kernel for attention-LSTM decoder (nn_Attention_49289044688898).

Data-parallel over batch: 512 rows -> 8 NeuronCores x 64 rows. Weights replicated.
Within a core, the 64 rows are split into TWO interleaved groups of 32 so the
attention spine of one group overlaps the LSTM tail of the other.

Per group g, per decode step s (26 steps):
  q   = h @ Wh                                  (PE, bh folded into Hproj)
  arg = HprojT + qT (broadcast over t)          (DVE, bf16 2x)
  th  = tanh(arg)                               (ACT)
  e   = sum_h Ws[h]*th[h, t, b]                 (PE, Ws stationary, col-groups)
  alpha = softmax_t(e)                          (DVE/ACT small)
  ctx = sum_t alpha[b,t]*batch_H[b,t,c]         (PE, block-diag alpha stationary)
  z   = ctx @ Kc + h @ R + onehot @ Ko'         (PE; Ko' has lstm_bias folded)
  gates (sigmoid via 0.5*tanh(x/2)+0.5) / c / h (ACT/DVE)
  probs[:, s, :] = h @ Wgen + bgen              (PE)
Layouts (per group, GB=32 rows):
  attention world: [128 part = h_lo, 4 h_hi, 64 t, 32 b]   (h = h_hi*128 + h_lo)
  context world:   [128 part = (b%2)*64 + t, 16 kt=b//2, 512 c]
  LSTM world:      [32 part = b, free]
"""

import os
import numpy as np
import ml_dtypes
from contextlib import ExitStack

B, T, C, H, NCC, S = 512, 64, 512, 512, 96, 26
NCORES = 8
BS = B // NCORES          # 64 batch rows per core
NG = 2                    # groups per core
GB = BS // NG             # 32 rows per group
BF = ml_dtypes.bfloat16

_CACHE = {}


def build_bass():
    import concourse.bass as bass
    import concourse.bacc as bacc
    import concourse.tile as tile
    import concourse.mybir as mybir

    f32 = mybir.dt.float32
    bf16 = mybir.dt.bfloat16
    AF = mybir.ActivationFunctionType
    AX = mybir.AxisListType

    nc = bacc.Bacc("TRN2", target_bir_lowering=False)

    # ---- DRAM I/O ----
    # bHT: [g, c, t, b32] ; bHc: [g, kt, (b2 t), c]
    bHT_d = nc.dram_tensor("bHT", [NG, C, T, GB], bf16, kind="ExternalInput")
    bHc_d = nc.dram_tensor("bHc", [NG, GB // 2, 128, C], bf16, kind="ExternalInput")
    wi_d = nc.dram_tensor("wi", [C, H], bf16, kind="ExternalInput")
    wh_d = nc.dram_tensor("wh", [H, H], bf16, kind="ExternalInput")
    bh_d = nc.dram_tensor("bh", [128, 4], f32, kind="ExternalInput")
    ws_d = nc.dram_tensor("ws", [128, 4, 32], bf16, kind="ExternalInput")
    kc_d = nc.dram_tensor("kc", [C, 4 * H], bf16, kind="ExternalInput")
    rr_d = nc.dram_tensor("rr", [H, 4 * H], bf16, kind="ExternalInput")
    ko_d = nc.dram_tensor("ko", [NCC, 4 * H], bf16, kind="ExternalInput")
    oh_d = nc.dram_tensor("oh", [NCC, S, BS], bf16, kind="ExternalInput")
    wg_d = nc.dram_tensor("wg", [H, NCC], bf16, kind="ExternalInput")
    bg_d = nc.dram_tensor("bg", [BS, NCC], f32, kind="ExternalInput")
    out_d = nc.dram_tensor("out", [BS, S, NCC], f32, kind="ExternalOutput")
    escr_d = nc.dram_tensor("escr", [NG, S, T * GB], f32)  # scratch for e scatter

    NCH = T * GB // 512  # 4 (t,b)-chunks of 512 per group

    with tile.TileContext(nc) as tc, ExitStack() as ctx:
        big = ctx.enter_context(tc.tile_pool(name="big", bufs=1))
        wpool = ctx.enter_context(tc.tile_pool(name="wpool", bufs=1))
        small = ctx.enter_context(tc.tile_pool(name="small", bufs=2))
        tiny = ctx.enter_context(tc.tile_pool(name="tiny", bufs=4))
        gates = ctx.enter_context(tc.tile_pool(name="gates", bufs=4))
        state = ctx.enter_context(tc.tile_pool(name="state", bufs=2))
        pzg = [ctx.enter_context(tc.tile_pool(name=f"pz{g}", bufs=1, space="PSUM"))
               for g in range(NG)]
        peg = [ctx.enter_context(tc.tile_pool(name=f"pe{g}", bufs=1, space="PSUM"))
               for g in range(NG)]
        pzj = ctx.enter_context(tc.tile_pool(name="pzj", bufs=3, space="PSUM"))

        dma = nc.sync
        import concourse.bass as _b

        # ---- load weights / big tensors ----
        bHc = [big.tile([128, GB // 2, C], bf16, tag=f"bHc{g}", name=f"bHc{g}") for g in range(NG)]
        for g in range(NG):
            dma.dma_start(out=bHc[g], in_=bHc_d[g].rearrange("k p c -> p k c"))
        # batch_H^T (prolog only; shares slots with tanh buffers)
        bHT = [big.tile([128, 4, T * GB], bf16, tag=f"th{g}", name=f"bHT{g}") for g in range(NG)]
        for g in range(NG):
            dma.dma_start(
                out=bHT[g],
                in_=bHT_d[g].rearrange("(ch cl) t b -> cl ch (t b)", cl=128))

        wi = wpool.tile([128, 4, H], bf16, tag="wi")
        dma.dma_start(out=wi, in_=wi_d[:].rearrange("(ch cl) h -> cl ch h", cl=128))
        wh = wpool.tile([128, 4, H], bf16, tag="wh")
        dma.dma_start(out=wh, in_=wh_d[:].rearrange("(hh hl) h -> hl hh h", hl=128))
        bh = wpool.tile([128, 4], f32, tag="bh")
        dma.dma_start(out=bh, in_=bh_d[:])
        ws = wpool.tile([128, 4, 32], bf16, tag="ws")
        dma.dma_start(out=ws, in_=ws_d[:])
        kc = wpool.tile([128, 4, 4 * H], bf16, tag="kc")
        dma.dma_start(out=kc, in_=kc_d[:].rearrange("(kh kl) n -> kl kh n", kl=128))
        rr = wpool.tile([128, 4, 4 * H], bf16, tag="rr")
        dma.dma_start(out=rr, in_=rr_d[:].rearrange("(kh kl) n -> kl kh n", kl=128))
        ko = wpool.tile([NCC, 4 * H], bf16, tag="ko")
        dma.dma_start(out=ko, in_=ko_d[:])
        oh = wpool.tile([NCC, S, BS], bf16, tag="oh")
        dma.dma_start(out=oh, in_=oh_d[:])
        wg = wpool.tile([128, 4, NCC], bf16, tag="wg")
        dma.dma_start(out=wg, in_=wg_d[:].rearrange("(hh hl) n -> hl hh n", hl=128))
        bg = wpool.tile([BS, NCC], f32, tag="bg")
        dma.dma_start(out=bg, in_=bg_d[:])

        # block-diag alpha holders (zeroed once)
        ablk = [wpool.tile([128, GB // 2, GB], bf16, tag=f"ablk{g}", name=f"ablk{g}")
                for g in range(NG)]
        for g in range(NG):
            nc.vector.memset(ablk[g], 0.0)

        # initial state (joint across groups)
        hTj = [state.tile([128, 4, BS], bf16, tag="hT", name="hT0")]
        nc.vector.memset(hTj[0], 0.0)
        c_stj = [state.tile([BS, H], f32, tag="c", name="c0")]
        nc.vector.memset(c_stj[0], 0.0)
        alpha_pad = [None] * NG
        for g in range(NG):
            alpha_pad[g] = small.tile([GB, 128], bf16, tag=f"apad{g}", name=f"apad{g}")
            nc.vector.memset(alpha_pad[g], 0.0)

        # ---- prolog: HprojT[g] = (batch_H @ Wi)^T + bh ----
        hprojT = [big.tile([128, 4, T * GB], bf16, tag=f"hp{g}", name=f"hp{g}") for g in range(NG)]
        for g in range(NG):
            for m in range(4):
                for n in range(NCH):
                    ps = pzg[g].tile([128, 512], f32, tag=f"pz{g}")
                    for k in range(4):
                        nc.tensor.matmul(
                            ps,
                            wi[:, k, m * 128:(m + 1) * 128],
                            bHT[g][:, k, n * 512:(n + 1) * 512],
                            start=(k == 0), stop=(k == 3),
                        )
                    nc.scalar.activation(
                        out=hprojT[g][:, m, n * 512:(n + 1) * 512], in_=ps,
                        func=AF.Identity, bias=bh[:, m:m + 1], scale=1.0,
                    )

        def bcast_t(ap2):
            # [128, GB(b)] -> [128, T(t, step0), GB(b)]
            return _b.AP(tensor=ap2.tensor, offset=ap2.offset,
                         ap=[ap2.ap[0], [0, T], ap2.ap[1]])

        # joint-LSTM state: hT holds BOTH groups' columns [128, 4, 64]
        # q matmul is joint (M=64); attention is per-group (b-halves of qT cols)

        def phase_q(s):
            # q = h @ Wh for all 64 rows -> qT [128, 4, 64]
            pq = pzj.tile([BS, H], f32, tag="pzj")
            for k in range(4):
                nc.tensor.matmul(pq, hTj[0][:, k, :], wh[:, k, :],
                                 start=(k == 0), stop=(k == 3))
            q_sb = small.tile([BS, H], bf16, tag="q_sb", bufs=2)
            nc.scalar.copy(q_sb, pq)
            qT = small.tile([128, 4, BS], bf16, tag="qT", bufs=2)
            for k in range(4):
                dma.dma_start(out=qT[:, k, :], in_=q_sb[:, k * 128:(k + 1) * 128],
                              transpose=True)
            return qT

        def phase_att(s, g, qT):
            gsl_b = slice(g * GB, (g + 1) * GB)
            th = big.tile([128, 4, T * GB], bf16, tag=f"th{g}", name=f"th{g}_{s}")
            pe = peg[g].tile([128, 512], f32, tag=f"pe{g}")
            for k in range(4):
                nc.vector.tensor_add(
                    th[:, k, :].rearrange("p (t b) -> p t b", t=T),
                    hprojT[g][:, k, :].rearrange("p (t b) -> p t b", t=T),
                    bcast_t(qT[:, k, gsl_b]))
                nc.scalar.activation(out=th[:, k, :], in_=th[:, k, :], func=AF.Tanh)
                for j in range(NCH):
                    bp = 32 * j
                    nc.tensor.matmul(pe[bp:bp + 32, :], ws[:, k, :],
                                     th[:, k, j * 512:(j + 1) * 512],
                                     start=(k == 0), stop=(k == 3),
                                     tile_position=(0, bp))
            est = small.tile([128, 512], f32, tag=f"est{g}", bufs=1,
                             name=f"est{g}_{s}")
            nc.vector.tensor_copy(est, pe)
            est_ap = est[:]
            src = _b.AP(tensor=est_ap.tensor, offset=est_ap.offset,
                        ap=[[est_ap.ap[0][0] * 32, 4], est_ap.ap[1]])
            dma.dma_start(out=escr_d[g, s, :], in_=src)
            e_sb = small.tile([GB, T], f32, tag=f"e_sb{g}", bufs=1,
                              name=f"e_sb{g}_{s}")
            esl = escr_d[g, s, :]
            src2 = _b.AP(tensor=esl.tensor, offset=esl.offset,
                         ap=[[1, GB], [GB, T]])
            dma.dma_start(out=e_sb, in_=src2)
            return e_sb

        def phase_post(s, g, e_sb, ctx_sb):
            # softmax over t, alpha scatter, ctx matmul, copy into joint ctx_sb
            mx = tiny.tile([GB, 1], f32, tag=f"mx{g}")
            nc.vector.reduce_max(mx, e_sb, axis=AX.X)
            nmx = tiny.tile([GB, 1], f32, tag=f"nmx{g}")
            nc.vector.tensor_scalar_mul(nmx, mx, -1.0)
            ex = small.tile([GB, T], f32, tag=f"ex{g}", bufs=1, name=f"ex{g}_{s}")
            nc.scalar.activation(out=ex, in_=e_sb, func=AF.Exp, bias=nmx, scale=1.0)
            sm = tiny.tile([GB, 1], f32, tag=f"sm{g}")
            nc.vector.reduce_sum(sm, ex, axis=AX.X)
            rcp = tiny.tile([GB, 1], f32, tag=f"rcp{g}")
            nc.vector.reciprocal(rcp, sm)
            nc.vector.tensor_scalar_mul(alpha_pad[g][:, 0:T], ex, rcp)
            alphaT = small.tile([128, GB], bf16, tag=f"alphaT{g}", bufs=2,
                                name=f"alphaT{g}_{s}")
            dma.dma_start(out=alphaT, in_=alpha_pad[g], transpose=True)
            aT = alphaT[:]
            ab = ablk[g][:]
            for par in (0, 1):
                srcp = _b.AP(tensor=aT.tensor, offset=aT.offset + par * aT.ap[1][0],
                             ap=[[aT.ap[0][0], T], [2 * aT.ap[1][0], GB // 2]])
                dst = _b.AP(tensor=ab.tensor,
                            offset=ab.offset + par * (64 * ab.ap[0][0] + ab.ap[2][0]),
                            ap=[[ab.ap[0][0], T], [ab.ap[1][0] + 2 * ab.ap[2][0], GB // 2]])
                dma.dma_start(out=dst, in_=srcp)
            pctx = pzg[g].tile([GB, C], f32, tag=f"pz{g}")
            for kt in range(GB // 2):
                nc.tensor.matmul(pctx, ablk[g][:, kt, :], bHc[g][:, kt, :],
                                 start=(kt == 0), stop=(kt == GB // 2 - 1))
            nc.scalar.copy(ctx_sb[g * GB:(g + 1) * GB, :], pctx)

        def phase_lstm(s, ctx_sb):
            # joint z for all 64 rows
            xTc = small.tile([128, 4, BS], bf16, tag="xTc", bufs=2,
                             name=f"xTc_{s}")
            for k in range(4):
                dma.dma_start(out=xTc[:, k, :], in_=ctx_sb[:, k * 128:(k + 1) * 128],
                              transpose=True)
            gate_sl = {"i": 0, "f": 1, "g": 2, "o": 3}
            sig = {}
            t1 = t2 = None
            for gname in ("f", "i", "g", "o"):
                zsl = slice(gate_sl[gname] * 512, (gate_sl[gname] + 1) * 512)
                pzt = pzj.tile([BS, 512], f32, tag="pzj")
                for k in range(4):
                    nc.tensor.matmul(pzt, xTc[:, k, :], kc[:, k, zsl],
                                     start=(k == 0), stop=False)
                for k in range(4):
                    nc.tensor.matmul(pzt, hTj[0][:, k, :], rr[:, k, zsl],
                                     start=False, stop=False)
                nc.tensor.matmul(pzt, oh[:, s, :], ko[:, zsl],
                                 start=False, stop=True)
                g_sb = gates.tile([BS, 512], f32, tag="gate", bufs=3)
                if gname == "g":
                    nc.scalar.activation(out=g_sb, in_=pzt, func=AF.Tanh)
                else:
                    nc.scalar.activation(out=g_sb, in_=pzt, func=AF.Tanh, scale=0.5)
                    nc.vector.tensor_scalar(out=g_sb, in0=g_sb,
                                            scalar1=0.5, scalar2=0.5,
                                            op0=mybir.AluOpType.mult,
                                            op1=mybir.AluOpType.add)
                sig[gname] = g_sb
                if gname == "f":
                    t1 = gates.tile([BS, H], f32, tag="tmp", bufs=2)
                    nc.vector.tensor_mul(t1, sig["f"], c_stj[0])
                elif gname == "g":
                    t2 = gates.tile([BS, H], f32, tag="tmp", bufs=2)
                    nc.vector.tensor_mul(t2, sig["i"], sig["g"])
                    c_stj[0] = state.tile([BS, H], f32, tag="c", name=f"c_{s}")
                    nc.vector.tensor_add(c_stj[0], t1, t2)
            tc_sb = gates.tile([BS, H], f32, tag="tmp", bufs=2)
            nc.scalar.activation(out=tc_sb, in_=c_stj[0], func=AF.Tanh)
            h_bf = small.tile([BS, H], bf16, tag="h_bf", bufs=1, name=f"h_bf_{s}")
            nc.vector.tensor_mul(h_bf, sig["o"], tc_sb)
            hTj[0] = state.tile([128, 4, BS], bf16, tag="hT", name=f"hT_{s}")
            for k in range(4):
                dma.dma_start(out=hTj[0][:, k, :],
                              in_=h_bf[:, k * 128:(k + 1) * 128], transpose=True)
            pp = peg[0].tile([128, 512], f32, tag="pe0")
            for k in range(4):
                nc.tensor.matmul(pp[0:BS, 0:NCC], hTj[0][:, k, :], wg[:, k, :],
                                 start=(k == 0), stop=(k == 3))
            pr_sb = small.tile([BS, NCC], f32, tag="pr_sb", bufs=2,
                               name=f"pr_{s}")
            nc.vector.tensor_add(pr_sb, pp[0:BS, 0:NCC], bg)
            dma.dma_start(out=out_d[:, s, :], in_=pr_sb)

        for s in range(S):
            qT = phase_q(s)
            e0 = phase_att(s, 0, qT)
            e1 = phase_att(s, 1, qT)
            ctx_sb = small.tile([BS, C], bf16, tag="ctx_sb", bufs=2,
                                name=f"ctx_{s}")
            phase_post(s, 0, e0, ctx_sb)
            phase_post(s, 1, e1, ctx_sb)
            phase_lstm(s, ctx_sb)

    nc.finalize()
    return nc


def _prep_core(inputs, i):
    bsl = slice(i * BS, (i + 1) * BS)
    bh_i = np.asarray(inputs["batch_H"][bsl], np.float32)          # [64, 64, 512]
    text_i = np.asarray(inputs["text"][bsl])                       # [64, 26]
    bh_g = bh_i.reshape(NG, GB, T, C)
    m = {}
    m["bHT"] = np.ascontiguousarray(bh_g.transpose(0, 3, 2, 1)).astype(BF)
    m["bHc"] = np.ascontiguousarray(bh_g.reshape(NG, GB // 2, 128, C)).astype(BF)
    m["wi"] = np.asarray(inputs["Wi"], np.float32).astype(BF)
    m["wh"] = np.asarray(inputs["Wh"], np.float32).astype(BF)
    m["bh"] = np.ascontiguousarray(
        np.asarray(inputs["bh"], np.float32).reshape(4, 128).T)
    wsr = np.ascontiguousarray(
        np.asarray(inputs["Ws"], np.float32)[:, 0].reshape(4, 128).T).astype(BF)
    m["ws"] = np.repeat(wsr[:, :, None], 32, axis=2)
    lk = np.asarray(inputs["lstm_kernel"], np.float32)
    lb = np.asarray(inputs["lstm_bias"], np.float32)
    m["kc"] = lk[:C].astype(BF)
    m["ko"] = (lk[C:] + lb[None, :]).astype(BF)
    m["rr"] = np.asarray(inputs["lstm_rec"], np.float32).astype(BF)
    m["oh"] = (np.arange(NCC)[:, None, None] == text_i.T[None, :, :]).astype(BF)
    m["wg"] = np.asarray(inputs["Wgen"], np.float32).astype(BF)
    m["bg"] = np.tile(np.asarray(inputs["bgen"], np.float32)[None, :], (BS, 1))
    return m


def kernel(_trace=False, **inputs):
    from concourse import bass_utils
    if "nc" not in _CACHE:
        _CACHE["nc"] = build_bass()
    nc = _CACHE["nc"]
    in_maps = [_prep_core(inputs, i) for i in range(NCORES)]
    res = bass_utils.run_bass_kernel_spmd(nc, in_maps, list(range(NCORES)),
                                          trace=_trace)
    _CACHE["last_result"] = res
    out = np.concatenate([r["out"] for r in res.results], axis=0)
    return out.astype(np.float32)



# revision 5
# speedup vs baseline: 1.0984x; 1.0984x over previous
"""Bass/Trainium2 kernel for attention-LSTM decoder (nn_Attention_49289044688898).

Data-parallel over batch: 512 rows -> 8 NeuronCores x 64 rows. Weights replicated.
Within a core, 64 rows = 2 groups of 32 for the attention; LSTM/q/probs joint.

v2 schedule (vs v1): no DMA transposes (PE transposes + direct-transposed qT
matmul), oh/h@R/probs matmuls hoisted into the tanh window, exp before the
e-scatter DRAM roundtrip with softmax normalization folded into the ctx
PSUM->SBUF copy, AF.Sigmoid for gates.

Per step s (26 steps):
  hT  = transpose(h)                        (PE, 4 transpose-mm)
  qT  = WhT-chunks @ hT                     (PE, 16 mm N=64, k-accum)
  probs(s-1) = hT-mm @ Wgen + bg            (PE + DVE, during tanh window)
  z-partial: onehot@Ko' + h@R               (PE, during tanh window)
  per group g: th = tanh(HprojT + qT)       (DVE add + ACT tanh, 4 chunks)
               e  = ws-quadrant mms         (PE)
               ex = exp(e) (PSUM->SBUF)     (ACT, no max-sub)
               scatter/gather via DRAM -> apad [b, t]
               sum+recip                    (DVE)
               alphaT = PE-transpose(apad); ablk scatter (2 DMA)
               ctx = ablk@bHc mms; scale by 1/sum in PSUM->SBUF copy
  xTc = PE-transpose(ctx)
  z  += xTc @ Kc                            (PE)
  gates: sigmoid/tanh (ACT) + c/h (DVE)
Layouts:
  attention world: [128 part = h_lo, 4 h_hi, 64 t, 32 b]
  context world:   [128 part = (b%2)*64 + t, 16 kt=b//2, 512 c]
  LSTM world:      [64 part = b, free]
"""

import numpy as np
import ml_dtypes
from contextlib import ExitStack

B, T, C, H, NCC, S = 512, 64, 512, 512, 96, 26
NCORES = 8
BS = B // NCORES          # 64 batch rows per core
NG = 2                    # groups per core
GB = BS // NG             # 32 rows per group
BF = ml_dtypes.bfloat16

_CACHE = {}


def build_bass():
    import concourse.bass as bass
    import concourse.bacc as bacc
    import concourse.tile as tile
    import concourse.mybir as mybir

    f32 = mybir.dt.float32
    bf16 = mybir.dt.bfloat16
    AF = mybir.ActivationFunctionType
    AX = mybir.AxisListType

    nc = bacc.Bacc("TRN2", target_bir_lowering=False)

    # ---- DRAM I/O ----
    bHT_d = nc.dram_tensor("bHT", [NG, C, T, GB], bf16, kind="ExternalInput")
    bHc_d = nc.dram_tensor("bHc", [NG, GB // 2, 128, C], bf16, kind="ExternalInput")
    wi_d = nc.dram_tensor("wi", [C, H], bf16, kind="ExternalInput")
    wh_d = nc.dram_tensor("wh", [H, H], bf16, kind="ExternalInput")
    bh_d = nc.dram_tensor("bh", [128, 4], f32, kind="ExternalInput")
    ws_d = nc.dram_tensor("ws", [128, 4, 32], bf16, kind="ExternalInput")
    kc_d = nc.dram_tensor("kc", [C, 4 * H], bf16, kind="ExternalInput")
    rr_d = nc.dram_tensor("rr", [H, 4 * H], bf16, kind="ExternalInput")
    ko_d = nc.dram_tensor("ko", [NCC, 4 * H], bf16, kind="ExternalInput")
    oh_d = nc.dram_tensor("oh", [NCC, S, BS], bf16, kind="ExternalInput")
    wg_d = nc.dram_tensor("wg", [H, NCC], bf16, kind="ExternalInput")
    bg_d = nc.dram_tensor("bg", [BS, NCC], f32, kind="ExternalInput")
    id_d = nc.dram_tensor("ident", [128, 128], bf16, kind="ExternalInput")
    out_d = nc.dram_tensor("out", [BS, S, NCC], f32, kind="ExternalOutput")
    escr_d = nc.dram_tensor("escr", [NG, S, T * GB], bf16)  # e-scatter scratch

    NCH = T * GB // 512  # 4 (t,b)-chunks of 512 per group

    with tile.TileContext(nc) as tc, ExitStack() as ctx:
        big = ctx.enter_context(tc.tile_pool(name="big", bufs=1))
        wpool = ctx.enter_context(tc.tile_pool(name="wpool", bufs=1))
        small = ctx.enter_context(tc.tile_pool(name="small", bufs=2))
        tiny = ctx.enter_context(tc.tile_pool(name="tiny", bufs=4))
        gates = ctx.enter_context(tc.tile_pool(name="gates", bufs=4))
        state = ctx.enter_context(tc.tile_pool(name="state", bufs=2))
        # PSUM pools (8 banks total):
        #   pz:  FI + GO gate accumulators  [128,512] x2     = 2 banks
        #   pep: e quadrant accumulator     [128,512] bufs=1 = 1 bank
        #   pcp: ctx accumulator            [32,512]  bufs=1 = 1 bank
        #   ptp: bf16 PE-transpose outs     [128,256] bufs=2 = 2 banks
        #   psm: qT/probs f32 matmul outs   [128,256] bufs=2 = 2 banks
        pz = ctx.enter_context(tc.tile_pool(name="pz", bufs=1, space="PSUM"))
        pep = ctx.enter_context(tc.tile_pool(name="pep", bufs=1, space="PSUM"))
        pcp = ctx.enter_context(tc.tile_pool(name="pcp", bufs=1, space="PSUM"))
        ptp = ctx.enter_context(tc.tile_pool(name="ptp", bufs=2, space="PSUM"))
        psm = ctx.enter_context(tc.tile_pool(name="psm", bufs=2, space="PSUM"))

        dma = nc.sync
        import concourse.bass as _b

        # ---- load weights / big tensors ----
        bHc = [big.tile([128, GB // 2, C], bf16, tag=f"bHc{g}", name=f"bHc{g}")
               for g in range(NG)]
        for g in range(NG):
            dma.dma_start(out=bHc[g], in_=bHc_d[g].rearrange("k p c -> p k c"))
        # batch_H^T (prolog only; shares slots with tanh buffers)
        bHT = [big.tile([128, 4, T * GB], bf16, tag=f"th{g}", name=f"bHT{g}")
               for g in range(NG)]
        for g in range(NG):
            dma.dma_start(
                out=bHT[g],
                in_=bHT_d[g].rearrange("(ch cl) t b -> cl ch (t b)", cl=128))

        wi = wpool.tile([128, 4, H], bf16, tag="wi")
        dma.dma_start(out=wi, in_=wi_d[:].rearrange("(ch cl) h -> cl ch h", cl=128))
        wh = wpool.tile([128, 4, H], bf16, tag="wh")
        dma.dma_start(out=wh, in_=wh_d[:].rearrange("(hh hl) h -> hl hh h", hl=128))
        bh = wpool.tile([128, 4], f32, tag="bh")
        dma.dma_start(out=bh, in_=bh_d[:])
        ws = wpool.tile([128, 4, 32], bf16, tag="ws")
        dma.dma_start(out=ws, in_=ws_d[:])
        kc = wpool.tile([128, 4, 4 * H], bf16, tag="kc")
        dma.dma_start(out=kc, in_=kc_d[:].rearrange("(kh kl) n -> kl kh n", kl=128))
        rr = wpool.tile([128, 4, 4 * H], bf16, tag="rr")
        dma.dma_start(out=rr, in_=rr_d[:].rearrange("(kh kl) n -> kl kh n", kl=128))
        ko = wpool.tile([NCC, 4 * H], bf16, tag="ko")
        dma.dma_start(out=ko, in_=ko_d[:])
        oh = wpool.tile([NCC, S, BS], bf16, tag="oh")
        dma.dma_start(out=oh, in_=oh_d[:])
        wg = wpool.tile([128, 4, NCC], bf16, tag="wg")
        dma.dma_start(out=wg, in_=wg_d[:].rearrange("(hh hl) n -> hl hh n", hl=128))
        bg = wpool.tile([BS, NCC], f32, tag="bg")
        dma.dma_start(out=bg, in_=bg_d[:])
        ident = wpool.tile([128, 128], bf16, tag="ident")
        dma.dma_start(out=ident, in_=id_d[:])

        # block-diag alpha holders (zeroed once)
        ablk = [wpool.tile([128, GB // 2, GB], bf16, tag=f"ablk{g}", name=f"ablk{g}")
                for g in range(NG)]
        for g in range(NG):
            nc.vector.memset(ablk[g], 0.0)
        # alpha pad holders [32, 128]: cols T..127 stay zero forever
        apad = [None] * NG
        for g in range(NG):
            apad[g] = wpool.tile([GB, 128], bf16, tag=f"apad{g}", name=f"apad{g}")
            nc.vector.memset(apad[g], 0.0)

        # initial state
        hT = [state.tile([128, 4, BS], bf16, tag="hT", name="hT0")]
        nc.vector.memset(hT[0], 0.0)
        c_st = [state.tile([BS, H], f32, tag="c", name="c0")]
        nc.vector.memset(c_st[0], 0.0)
        hbf = [None]

        # ---- prolog: HprojT[g] = (batch_H @ Wi)^T + bh ----
        hprojT = [big.tile([128, 4, T * GB], bf16, tag=f"hp{g}", name=f"hp{g}")
                  for g in range(NG)]
        for g in range(NG):
            for m in range(4):
                for n in range(NCH):
                    ps = pz.tile([128, 512], f32, tag="FI" if g == 0 else "GO")
                    for k in range(4):
                        nc.tensor.matmul(
                            ps,
                            wi[:, k, m * 128:(m + 1) * 128],
                            bHT[g][:, k, n * 512:(n + 1) * 512],
                            start=(k == 0), stop=(k == 3),
                        )
                    nc.scalar.activation(
                        out=hprojT[g][:, m, n * 512:(n + 1) * 512], in_=ps,
                        func=AF.Identity, bias=bh[:, m:m + 1], scale=1.0,
                    )

        def bcast_t(ap2):
            # [128, GB(b)] -> [128, T(t, stride0), GB(b)]
            return _b.AP(tensor=ap2.tensor, offset=ap2.offset,
                         ap=[ap2.ap[0], [0, T], ap2.ap[1]])

        gate_sl = {"f": 1, "i": 0, "g": 2, "o": 3}
        # gate -> (psum tag, row offset): f/i share FI bank, g/o share GO bank
        gate_loc = {"f": ("FI", 0), "i": ("FI", 64), "g": ("GO", 0), "o": ("GO", 64)}

        def emit_hT_transpose(s):
            # h_bf [64, 512] -> hT [128, 4, 64] via 4 PE transposes
            phT = ptp.tile([128, 256], bf16, tag="tp", name=f"phT_{s}")
            for m in range(4):
                nc.tensor.transpose(phT[:, m * 64:(m + 1) * 64],
                                    hbf[0][:, m * 128:(m + 1) * 128],
                                    ident[0:BS, 0:BS])
            hT[0] = state.tile([128, 4, BS], bf16, tag="hT", name=f"hT_{s}")
            nc.vector.tensor_copy(hT[0], phT)

        def emit_qT(s):
            # qT[h',b] = sum_h Wh[h,h'] hT[h,b]; m-outer so chunk m is
            # copied out as soon as its k-accumulation finishes.
            pqT = psm.tile([128, 256], f32, tag="pq", name=f"pqT_{s}")
            qT = small.tile([128, 4, BS], bf16, tag="qT", bufs=2, name=f"qT_{s}")
            for m in range(4):
                for k in range(4):
                    nc.tensor.matmul(pqT[:, m * 64:(m + 1) * 64],
                                     wh[:, k, m * 128:(m + 1) * 128],
                                     hT[0][:, k, :],
                                     start=(k == 0), stop=(k == 3))
                nc.vector.tensor_copy(qT[:, m, :], pqT[:, m * 64:(m + 1) * 64])
            return qT

        def emit_probs(sm1):
            # probs(sm1) = h(sm1) @ Wgen + bg, from hT
            pp = psm.tile([128, 256], f32, tag="pq", name=f"pp_{sm1}")
            for k in range(4):
                nc.tensor.matmul(pp[0:BS, 0:NCC], hT[0][:, k, :], wg[:, k, :],
                                 start=(k == 0), stop=(k == 3))
            pr = small.tile([BS, NCC], f32, tag="pr", bufs=2, name=f"pr_{sm1}")
            nc.vector.tensor_add(pr, pp[0:BS, 0:NCC], bg)
            dma.dma_start(out=out_d[:, sm1, :], in_=pr)

        def emit_z_early(s, pzt):
            # onehot@Ko' (start) + h@R during the tanh window
            for gn in "figo":
                tag, ro = gate_loc[gn]
                zsl = slice(gate_sl[gn] * 512, (gate_sl[gn] + 1) * 512)
                nc.tensor.matmul(pzt[tag][ro:ro + 64, :], oh[:, s, :],
                                 ko[:, zsl], start=True, stop=False,
                                 tile_position=(0, ro))
            for k in range(4):
                for gn in "figo":
                    tag, ro = gate_loc[gn]
                    zsl = slice(gate_sl[gn] * 512, (gate_sl[gn] + 1) * 512)
                    nc.tensor.matmul(pzt[tag][ro:ro + 64, :], hT[0][:, k, :],
                                     rr[:, k, zsl], start=False, stop=False,
                                     tile_position=(0, ro))

        def emit_att_tanh(s, g, qT, pe_):
            # DVE add + ACT tanh + e quadrant mms for group g
            gsl_b = slice(g * GB, (g + 1) * GB)
            th = big.tile([128, 4, T * GB], bf16, tag=f"th{g}", name=f"th{g}_{s}")
            for k in range(4):
                nc.vector.tensor_add(
                    th[:, k, :].rearrange("p (t b) -> p t b", t=T),
                    hprojT[g][:, k, :].rearrange("p (t b) -> p t b", t=T),
                    bcast_t(qT[:, k, gsl_b]))
                nc.scalar.activation(out=th[:, k, :], in_=th[:, k, :], func=AF.Tanh)
                for j in range(NCH):
                    bp = 32 * j
                    nc.tensor.matmul(pe_[bp:bp + 32, :], ws[:, k, :],
                                     th[:, k, j * 512:(j + 1) * 512],
                                     start=(k == 0), stop=(k == 3),
                                     tile_position=(0, bp))

        def emit_exp_scatter(s, g, pe_):
            # exp on the PSUM layout, then DRAM roundtrip to [b, t]
            est = small.tile([128, 512], bf16, tag=f"est{g}", bufs=1,
                             name=f"est{g}_{s}")
            nc.scalar.activation(out=est, in_=pe_, func=AF.Exp)
            est_ap = est[:]
            src = _b.AP(tensor=est_ap.tensor, offset=est_ap.offset,
                        ap=[[est_ap.ap[0][0] * 32, 4], est_ap.ap[1]])
            dma.dma_start(out=escr_d[g, s, :], in_=src)
            esl = escr_d[g, s, :]
            src2 = _b.AP(tensor=esl.tensor, offset=esl.offset,
                         ap=[[1, GB], [GB, T]])
            dma.dma_start(out=apad[g][:, 0:T], in_=src2)

        def emit_post(s, g, ctx_sb):
            # ctx_sb: per-group [GB, C] tile (base partition 0)
            # sum/recip, alphaT via PE transpose, ablk scatter, ctx mms,
            # normalize in the PSUM->SBUF copy
            sm = tiny.tile([GB, 1], f32, tag=f"sm{g}")
            nc.vector.reduce_sum(sm, apad[g][:, 0:T], axis=AX.X)
            rcp = tiny.tile([GB, 1], f32, tag=f"rcp{g}")
            nc.vector.reciprocal(rcp, sm)
            paT = ptp.tile([128, 256], bf16, tag="tp", name=f"paT{g}_{s}")
            nc.tensor.transpose(paT[:, 0:GB], apad[g], ident[0:GB, 0:GB])
            alphaT = small.tile([128, GB], bf16, tag=f"alphaT{g}", bufs=2,
                                name=f"alphaT{g}_{s}")
            nc.vector.tensor_copy(alphaT, paT[:, 0:GB])
            aT = alphaT[:]
            ab = ablk[g][:]
            for par in (0, 1):
                srcp = _b.AP(tensor=aT.tensor, offset=aT.offset + par * aT.ap[1][0],
                             ap=[[aT.ap[0][0], T], [2 * aT.ap[1][0], GB // 2]])
                dst = _b.AP(tensor=ab.tensor,
                            offset=ab.offset + par * (64 * ab.ap[0][0] + ab.ap[2][0]),
                            ap=[[ab.ap[0][0], T], [ab.ap[1][0] + 2 * ab.ap[2][0], GB // 2]])
                dma.dma_start(out=dst, in_=srcp)
            pctx = pcp.tile([GB, C], f32, tag="ctx")
            for kt in range(GB // 2):
                nc.tensor.matmul(pctx, ablk[g][:, kt, :], bHc[g][:, kt, :],
                                 start=(kt == 0), stop=(kt == GB // 2 - 1))
            nc.vector.tensor_scalar_mul(ctx_sb, pctx, rcp)

        def emit_ctxT(s, g, ctx_sb, pxT, xTc):
            # ctx rows of group g -> xTc[:, k, g*32:(g+1)*32]
            for k in range(4):
                nc.tensor.transpose(pxT[g][:, k * GB:(k + 1) * GB],
                                    ctx_sb[:, k * 128:(k + 1) * 128],
                                    ident[0:GB, 0:GB])
            src = pxT[g][:, 0:128].rearrange("p (k b) -> p k b", k=4)
            nc.vector.tensor_copy(xTc[:, :, g * GB:(g + 1) * GB], src)

        def emit_z_late(s, pzt, xTc):
            for k in range(4):
                for gn in "figo":
                    tag, ro = gate_loc[gn]
                    zsl = slice(gate_sl[gn] * 512, (gate_sl[gn] + 1) * 512)
                    nc.tensor.matmul(pzt[tag][ro:ro + 64, :], xTc[:, k, :],
                                     kc[:, k, zsl], start=False, stop=(k == 3),
                                     tile_position=(0, ro))

        def emit_gates(s, pzt):
            sig = {}
            t1 = t2 = None
            for gn in ("f", "i", "g", "o"):
                tag, ro = gate_loc[gn]
                g_sb = gates.tile([BS, H], f32, tag="gate", bufs=4)
                if gn == "g":
                    nc.scalar.activation(out=g_sb, in_=pzt[tag][ro:ro + 64, :],
                                         func=AF.Tanh)
                else:
                    # sigmoid via tanh: keeps ACT on the exp_and_others table
                    nc.scalar.activation(out=g_sb, in_=pzt[tag][ro:ro + 64, :],
                                         func=AF.Tanh, scale=0.5)
                    nc.vector.tensor_scalar(out=g_sb, in0=g_sb,
                                            scalar1=0.5, scalar2=0.5,
                                            op0=mybir.AluOpType.mult,
                                            op1=mybir.AluOpType.add)
                sig[gn] = g_sb
                if gn == "f":
                    t1 = gates.tile([BS, H], f32, tag="tmp", bufs=2)
                    nc.vector.tensor_mul(t1, sig["f"], c_st[0])
                elif gn == "g":
                    t2 = gates.tile([BS, H], f32, tag="tmp", bufs=2)
                    nc.vector.tensor_mul(t2, sig["i"], sig["g"])
                    c_st[0] = state.tile([BS, H], f32, tag="c", name=f"c_{s}")
                    nc.vector.tensor_add(c_st[0], t1, t2)
            tc_sb = gates.tile([BS, H], f32, tag="tmp", bufs=2)
            nc.scalar.activation(out=tc_sb, in_=c_st[0], func=AF.Tanh)
            hbf[0] = small.tile([BS, H], bf16, tag="h_bf", bufs=2,
                                name=f"h_bf_{s}")
            nc.vector.tensor_mul(hbf[0], sig["o"], tc_sb)

        for s in range(S):
            if s > 0:
                emit_hT_transpose(s)
            qT = emit_qT(s)
            if s > 0:
                emit_probs(s - 1)
            pzt = {"FI": pz.tile([128, 512], f32, tag="FI", name=f"pzFI_{s}"),
                   "GO": pz.tile([128, 512], f32, tag="GO", name=f"pzGO_{s}")}
            emit_z_early(s, pzt)
            pe_ = [None] * NG
            ctx_sb = [small.tile([GB, C], bf16, tag=f"ctx_sb{g}", bufs=2,
                                 name=f"ctx{g}_{s}") for g in range(NG)]
            pxT = [None] * NG
            xTc = small.tile([128, 4, BS], bf16, tag="xTc", bufs=2,
                             name=f"xTc_{s}")
            for g in range(NG):
                pe_[g] = pep.tile([128, 512], f32, tag="pe", name=f"pe{g}_{s}")
                emit_att_tanh(s, g, qT, pe_[g])
                emit_exp_scatter(s, g, pe_[g])
            for g in range(NG):
                emit_post(s, g, ctx_sb[g])
                pxT[g] = ptp.tile([128, 256], bf16, tag="tp", name=f"pxT{g}_{s}")
                emit_ctxT(s, g, ctx_sb[g], pxT, xTc)
            emit_z_late(s, pzt, xTc)
            emit_gates(s, pzt)
        emit_hT_transpose(S)
        emit_probs(S - 1)

    nc.finalize()
    return nc


def _prep_core(inputs, i):
    bsl = slice(i * BS, (i + 1) * BS)
    bh_i = np.asarray(inputs["batch_H"][bsl], np.float32)          # [64, 64, 512]
    text_i = np.asarray(inputs["text"][bsl])                       # [64, 26]
    bh_g = bh_i.reshape(NG, GB, T, C)
    m = {}
    m["bHT"] = np.ascontiguousarray(bh_g.transpose(0, 3, 2, 1)).astype(BF)
    m["bHc"] = np.ascontiguousarray(bh_g.reshape(NG, GB // 2, 128, C)).astype(BF)
    m["wi"] = np.asarray(inputs["Wi"], np.float32).astype(BF)
    m["wh"] = np.asarray(inputs["Wh"], np.float32).astype(BF)
    m["bh"] = np.ascontiguousarray(
        np.asarray(inputs["bh"], np.float32).reshape(4, 128).T)
    wsr = np.ascontiguousarray(
        np.asarray(inputs["Ws"], np.float32)[:, 0].reshape(4, 128).T).astype(BF)
    m["ws"] = np.repeat(wsr[:, :, None], 32, axis=2)
    lk = np.asarray(inputs["lstm_kernel"], np.float32)
    lb = np.asarray(inputs["lstm_bias"], np.float32)
    m["kc"] = lk[:C].astype(BF)
    m["ko"] = (lk[C:] + lb[None, :]).astype(BF)
    m["rr"] = np.asarray(inputs["lstm_rec"], np.float32).astype(BF)
    m["oh"] = (np.arange(NCC)[:, None, None] == text_i.T[None, :, :]).astype(BF)
    m["wg"] = np.asarray(inputs["Wgen"], np.float32).astype(BF)
    m["bg"] = np.tile(np.asarray(inputs["bgen"], np.float32)[None, :], (BS, 1))
    m["ident"] = np.eye(128, dtype=np.float32).astype(BF)
    return m


def kernel(_trace=False, **inputs):
    from concourse import bass_utils
    if "nc" not in _CACHE:
        _CACHE["nc"] = build_bass()
    nc = _CACHE["nc"]
    in_maps = [_prep_core(inputs, i) for i in range(NCORES)]
    res = bass_utils.run_bass_kernel_spmd(nc, in_maps, list(range(NCORES)),
                                          trace=_trace)
    _CACHE["last_result"] = res
    out = np.concatenate([r["out"] for r in res.results], axis=0)
    return out.astype(np.float32)


# revision 7
# speedup vs baseline: 1.5641x; 1.4240x over previous
"""Bass/Trainium2 kernel for attention-LSTM decoder (nn_Attention_49289044688898).

Data-parallel over batch: 512 rows -> 8 NeuronCores x 64 rows. Weights replicated.
Within a core, 64 rows = 2 groups of 32 for the attention; LSTM/q/probs joint.

v2 schedule (vs v1): no DMA transposes (PE transposes + direct-transposed qT
matmul), oh/h@R/probs matmuls hoisted into the tanh window, exp before the
e-scatter DRAM roundtrip with softmax normalization folded into the ctx
PSUM->SBUF copy, AF.Sigmoid for gates.

Per step s (26 steps):
  hT  = transpose(h)                        (PE, 4 transpose-mm)
  qT  = WhT-chunks @ hT                     (PE, 16 mm N=64, k-accum)
  probs(s-1) = hT-mm @ Wgen + bg            (PE + DVE, during tanh window)
  z-partial: onehot@Ko' + h@R               (PE, during tanh window)
  per group g: th = tanh(HprojT + qT)       (DVE add + ACT tanh, 4 chunks)
               e  = ws-quadrant mms         (PE)
               ex = exp(e) (PSUM->SBUF)     (ACT, no max-sub)
               scatter/gather via DRAM -> apad [b, t]
               sum+recip                    (DVE)
               alphaT = PE-transpose(apad); ablk scatter (2 DMA)
               ctx = ablk@bHc mms; scale by 1/sum in PSUM->SBUF copy
  xTc = PE-transpose(ctx)
  z  += xTc @ Kc                            (PE)
  gates: sigmoid/tanh (ACT) + c/h (DVE)
Layouts:
  attention world: [128 part = h_lo, 4 h_hi, 64 t, 32 b]
  context world:   [128 part = (b%2)*64 + t, 16 kt=b//2, 512 c]
  LSTM world:      [64 part = b, free]
"""

import numpy as np
import ml_dtypes
from contextlib import ExitStack

B, T, C, H, NCC, S = 512, 64, 512, 512, 96, 26
NCORES = 8
BS = B // NCORES          # 64 batch rows per core
NG = 2                    # groups per core
GB = BS // NG             # 32 rows per group
BF = ml_dtypes.bfloat16

_CACHE = {}


def build_bass():
    import concourse.bass as bass
    import concourse.bacc as bacc
    import concourse.tile as tile
    import concourse.mybir as mybir

    f32 = mybir.dt.float32
    bf16 = mybir.dt.bfloat16
    AF = mybir.ActivationFunctionType
    AX = mybir.AxisListType

    nc = bacc.Bacc("TRN2", target_bir_lowering=False)

    # ---- DRAM I/O ----
    bHT_d = nc.dram_tensor("bHT", [NG, C, T, GB], bf16, kind="ExternalInput")
    bHc_d = nc.dram_tensor("bHc", [NG, GB // 2, 128, C], bf16, kind="ExternalInput")
    wi_d = nc.dram_tensor("wi", [C, H], bf16, kind="ExternalInput")
    wh_d = nc.dram_tensor("wh", [H, H], bf16, kind="ExternalInput")
    bh_d = nc.dram_tensor("bh", [128, 4], f32, kind="ExternalInput")
    ws_d = nc.dram_tensor("ws", [128, 4, 32], bf16, kind="ExternalInput")
    kc_d = nc.dram_tensor("kc", [C, 4 * H], bf16, kind="ExternalInput")
    rr_d = nc.dram_tensor("rr", [H, 4 * H], bf16, kind="ExternalInput")
    ko_d = nc.dram_tensor("ko", [NCC, 4 * H], bf16, kind="ExternalInput")
    oh_d = nc.dram_tensor("oh", [NCC, S, BS], bf16, kind="ExternalInput")
    wg_d = nc.dram_tensor("wg", [H, NCC], bf16, kind="ExternalInput")
    bg_d = nc.dram_tensor("bg", [BS, NCC], f32, kind="ExternalInput")
    id_d = nc.dram_tensor("ident", [128, 128], bf16, kind="ExternalInput")
    out_d = nc.dram_tensor("out", [BS, S, NCC], f32, kind="ExternalOutput")

    NCH = T * GB // 512  # 4 (t,b)-chunks of 512 per group

    with tile.TileContext(nc) as tc, ExitStack() as ctx:
        big = ctx.enter_context(tc.tile_pool(name="big", bufs=1))
        wpool = ctx.enter_context(tc.tile_pool(name="wpool", bufs=1))
        small = ctx.enter_context(tc.tile_pool(name="small", bufs=2))
        tiny = ctx.enter_context(tc.tile_pool(name="tiny", bufs=4))
        gates = ctx.enter_context(tc.tile_pool(name="gates", bufs=4))
        state = ctx.enter_context(tc.tile_pool(name="state", bufs=2))
        # PSUM pools (8 banks total):
        #   pz:  FI + GO gate accumulators  [128,512] x2     = 2 banks
        #   pep: e quadrant accumulator     [128,512] bufs=2 = 2 banks
        #   pcp: ctx accumulator            [32,512]  bufs=1 = 1 bank
        #   ptp: bf16 PE-transpose outs     [128,256] bufs=1 = 1 bank
        #   psm: qT/probs/sums f32 mm outs  [128,256] bufs=2 = 2 banks
        pz = ctx.enter_context(tc.tile_pool(name="pz", bufs=1, space="PSUM"))
        pep = ctx.enter_context(tc.tile_pool(name="pep", bufs=2, space="PSUM"))
        pcp = ctx.enter_context(tc.tile_pool(name="pcp", bufs=1, space="PSUM"))
        ptp = ctx.enter_context(tc.tile_pool(name="ptp", bufs=1, space="PSUM"))
        psm = ctx.enter_context(tc.tile_pool(name="psm", bufs=2, space="PSUM"))

        dma = nc.sync
        import concourse.bass as _b

        # ---- load weights / big tensors ----
        bHc = [big.tile([128, GB // 2, C], bf16, tag=f"bHc{g}", name=f"bHc{g}")
               for g in range(NG)]
        for g in range(NG):
            dma.dma_start(out=bHc[g], in_=bHc_d[g].rearrange("k p c -> p k c"))
        # batch_H^T (prolog only; shares slots with tanh buffers)
        bHT = [big.tile([128, 4, T * GB], bf16, tag=f"th{g}", name=f"bHT{g}")
               for g in range(NG)]
        for g in range(NG):
            dma.dma_start(
                out=bHT[g],
                in_=bHT_d[g].rearrange("(ch cl) t b -> cl ch (t b)", cl=128))

        wi = wpool.tile([128, 4, H], bf16, tag="wi")
        dma.dma_start(out=wi, in_=wi_d[:].rearrange("(ch cl) h -> cl ch h", cl=128))
        wh = wpool.tile([128, 4, H], bf16, tag="wh")
        dma.dma_start(out=wh, in_=wh_d[:].rearrange("(hh hl) h -> hl hh h", hl=128))
        bh = wpool.tile([128, 4], f32, tag="bh")
        dma.dma_start(out=bh, in_=bh_d[:])
        ws = wpool.tile([128, 4, 32], bf16, tag="ws")
        dma.dma_start(out=ws, in_=ws_d[:])
        kc = wpool.tile([128, 4, 4 * H], bf16, tag="kc")
        dma.dma_start(out=kc, in_=kc_d[:].rearrange("(kh kl) n -> kl kh n", kl=128))
        rr = wpool.tile([128, 4, 4 * H], bf16, tag="rr")
        dma.dma_start(out=rr, in_=rr_d[:].rearrange("(kh kl) n -> kl kh n", kl=128))
        ko = wpool.tile([NCC, 4 * H], bf16, tag="ko")
        dma.dma_start(out=ko, in_=ko_d[:])
        oh = wpool.tile([NCC, S, BS], bf16, tag="oh")
        dma.dma_start(out=oh, in_=oh_d[:])
        wg = wpool.tile([128, 4, NCC], bf16, tag="wg")
        dma.dma_start(out=wg, in_=wg_d[:].rearrange("(hh hl) n -> hl hh n", hl=128))
        bg = wpool.tile([BS, NCC], f32, tag="bg")
        dma.dma_start(out=bg, in_=bg_d[:])
        ident = wpool.tile([128, 128], bf16, tag="ident")
        dma.dma_start(out=ident, in_=id_d[:])
        ones = wpool.tile([T, 1], bf16, tag="ones")
        nc.vector.memset(ones, 1.0)

        # block-diag alpha holders (zeroed once)
        ablk = [wpool.tile([128, GB // 2, GB], bf16, tag=f"ablk{g}", name=f"ablk{g}")
                for g in range(NG)]
        for g in range(NG):
            nc.vector.memset(ablk[g], 0.0)

        # initial state
        hT = [state.tile([128, 4, BS], bf16, tag="hT", name="hT0")]
        nc.vector.memset(hT[0], 0.0)
        c_st = [state.tile([BS, H], f32, tag="c", name="c0")]
        nc.vector.memset(c_st[0], 0.0)
        hbf = [None]

        # ---- prolog: HprojT[g] = (batch_H @ Wi)^T + bh ----
        hprojT = [big.tile([128, 4, T * GB], bf16, tag=f"hp{g}", name=f"hp{g}")
                  for g in range(NG)]
        for g in range(NG):
            for m in range(4):
                for n in range(NCH):
                    ps = pz.tile([128, 512], f32, tag="FI" if g == 0 else "GO")
                    for k in range(4):
                        nc.tensor.matmul(
                            ps,
                            wi[:, k, m * 128:(m + 1) * 128],
                            bHT[g][:, k, n * 512:(n + 1) * 512],
                            start=(k == 0), stop=(k == 3),
                        )
                    nc.scalar.activation(
                        out=hprojT[g][:, m, n * 512:(n + 1) * 512], in_=ps,
                        func=AF.Identity, bias=bh[:, m:m + 1], scale=1.0,
                    )

        def bcast_t(ap2):
            # [128, GB(b)] -> [128, T(t, stride0), GB(b)]
            return _b.AP(tensor=ap2.tensor, offset=ap2.offset,
                         ap=[ap2.ap[0], [0, T], ap2.ap[1]])

        gate_sl = {"f": 1, "i": 0, "g": 2, "o": 3}
        # gate -> (psum tag, row offset): f/i share FI bank, g/o share GO bank
        gate_loc = {"f": ("FI", 0), "i": ("FI", 64), "g": ("GO", 0), "o": ("GO", 64)}

        def emit_hT_transpose(s):
            # h_bf [64, 512] -> hT [128, 4, 64] via 4 PE transposes
            phT = ptp.tile([128, 256], bf16, tag="tp", name=f"phT_{s}")
            for m in range(4):
                nc.tensor.transpose(phT[:, m * 64:(m + 1) * 64],
                                    hbf[0][:, m * 128:(m + 1) * 128],
                                    ident[0:BS, 0:BS])
            hT[0] = state.tile([128, 4, BS], bf16, tag="hT", name=f"hT_{s}")
            nc.vector.tensor_copy(hT[0], phT)

        def emit_qT(s):
            # qT[h',b] = sum_h Wh[h,h'] hT[h,b]; m-outer so chunk m is
            # copied out as soon as its k-accumulation finishes.
            pqT = psm.tile([128, 256], f32, tag="pq", name=f"pqT_{s}")
            qT = small.tile([128, 4, BS], bf16, tag="qT", bufs=2, name=f"qT_{s}")
            for m in range(4):
                for k in range(4):
                    nc.tensor.matmul(pqT[:, m * 64:(m + 1) * 64],
                                     wh[:, k, m * 128:(m + 1) * 128],
                                     hT[0][:, k, :],
                                     start=(k == 0), stop=(k == 3))
                nc.vector.tensor_copy(qT[:, m, :], pqT[:, m * 64:(m + 1) * 64])
            return qT

        def emit_probs(sm1):
            # probs(sm1) = h(sm1) @ Wgen + bg, from hT
            pp = psm.tile([128, 256], f32, tag="pq", name=f"pp_{sm1}")
            for k in range(4):
                nc.tensor.matmul(pp[0:BS, 0:NCC], hT[0][:, k, :], wg[:, k, :],
                                 start=(k == 0), stop=(k == 3))
            pr = small.tile([BS, NCC], f32, tag="pr", bufs=2, name=f"pr_{sm1}")
            nc.vector.tensor_add(pr, pp[0:BS, 0:NCC], bg)
            dma.dma_start(out=out_d[:, sm1, :], in_=pr)

        def emit_z_early(s, pzt):
            # onehot@Ko' (start) + h@R during the tanh window
            for gn in "figo":
                tag, ro = gate_loc[gn]
                zsl = slice(gate_sl[gn] * 512, (gate_sl[gn] + 1) * 512)
                nc.tensor.matmul(pzt[tag][ro:ro + 64, :], oh[:, s, :],
                                 ko[:, zsl], start=True, stop=False,
                                 tile_position=(0, ro))
            for k in range(4):
                for gn in "figo":
                    tag, ro = gate_loc[gn]
                    zsl = slice(gate_sl[gn] * 512, (gate_sl[gn] + 1) * 512)
                    nc.tensor.matmul(pzt[tag][ro:ro + 64, :], hT[0][:, k, :],
                                     rr[:, k, zsl], start=False, stop=False,
                                     tile_position=(0, ro))

        def emit_att_tanh(s, g, qT, pe_):
            # DVE add + ACT tanh + e quadrant mms for group g
            gsl_b = slice(g * GB, (g + 1) * GB)
            th = big.tile([128, 4, T * GB], bf16, tag=f"th{g}", name=f"th{g}_{s}")
            for k in range(4):
                nc.vector.tensor_add(
                    th[:, k, :].rearrange("p (t b) -> p t b", t=T),
                    hprojT[g][:, k, :].rearrange("p (t b) -> p t b", t=T),
                    bcast_t(qT[:, k, gsl_b]))
                nc.scalar.activation(out=th[:, k, :], in_=th[:, k, :], func=AF.Tanh)
                for j in range(NCH):
                    bp = 32 * j
                    nc.tensor.matmul(pe_[bp:bp + 32, :], ws[:, k, :],
                                     th[:, k, j * 512:(j + 1) * 512],
                                     start=(k == 0), stop=(k == 3),
                                     tile_position=(0, bp))

        def emit_exp_scatter(s, g, pe_):
            # exp on the PSUM layout; est row 32j is exactly alphaT rows
            # 16j..16j+16 as a [16, 32] block -> 4 row-spread SBUF DMAs
            est = small.tile([128, 512], bf16, tag=f"est{g}", bufs=1,
                             name=f"est{g}_{s}")
            nc.scalar.activation(out=est, in_=pe_, func=AF.Exp)
            alphaT = small.tile([T, GB], bf16, tag=f"alphaT{g}", bufs=2,
                                name=f"alphaT{g}_{s}")
            for j in range(4):
                esl = est[32 * j:32 * j + 1, :]
                srcj = _b.AP(tensor=esl.tensor, offset=esl.offset,
                             ap=[[esl.ap[0][0], 1], [GB, T // 4], [1, GB]])
                nc.gpsimd.dma_start(out=alphaT[16 * j:16 * (j + 1), :], in_=srcj)
            return alphaT

        def emit_post(s, g, ctx_sb, alphaT):
            # ctx_sb: per-group [GB, C] tile (base partition 0)
            # denominator: sums[b] = alphaT^T @ ones  (one matmul, N=1)
            psums = psm.tile([128, 256], f32, tag="pq", name=f"psm{g}_{s}")
            nc.tensor.matmul(psums[0:GB, 0:1], alphaT, ones,
                             start=True, stop=True)
            rcp = tiny.tile([GB, 1], f32, tag=f"rcp{g}")
            nc.vector.reciprocal(rcp, psums[0:GB, 0:1])
            aT = alphaT[:]
            ab = ablk[g][:]
            for par in (0, 1):
                srcp = _b.AP(tensor=aT.tensor, offset=aT.offset + par * aT.ap[1][0],
                             ap=[[aT.ap[0][0], T], [2 * aT.ap[1][0], GB // 2]])
                dst = _b.AP(tensor=ab.tensor,
                            offset=ab.offset + par * (64 * ab.ap[0][0] + ab.ap[2][0]),
                            ap=[[ab.ap[0][0], T], [ab.ap[1][0] + 2 * ab.ap[2][0], GB // 2]])
                dma.dma_start(out=dst, in_=srcp)
            pctx = pcp.tile([GB, C], f32, tag="ctx")
            for kt in range(GB // 2):
                nc.tensor.matmul(pctx, ablk[g][:, kt, :], bHc[g][:, kt, :],
                                 start=(kt == 0), stop=(kt == GB // 2 - 1))
            nc.vector.tensor_scalar_mul(ctx_sb, pctx, rcp)

        def emit_ctxT(s, g, ctx_sb, pxT, xTc):
            # ctx rows of group g -> xTc[:, k, g*32:(g+1)*32]
            for k in range(4):
                nc.tensor.transpose(pxT[g][:, k * GB:(k + 1) * GB],
                                    ctx_sb[:, k * 128:(k + 1) * 128],
                                    ident[0:GB, 0:GB])
            src = pxT[g][:, 0:128].rearrange("p (k b) -> p k b", k=4)
            nc.vector.tensor_copy(xTc[:, :, g * GB:(g + 1) * GB], src)

        def emit_z_late(s, pzt, xTc):
            for k in range(4):
                for gn in "figo":
                    tag, ro = gate_loc[gn]
                    zsl = slice(gate_sl[gn] * 512, (gate_sl[gn] + 1) * 512)
                    nc.tensor.matmul(pzt[tag][ro:ro + 64, :], xTc[:, k, :],
                                     kc[:, k, zsl], start=False, stop=(k == 3),
                                     tile_position=(0, ro))

        def emit_gates(s, pzt):
            sig = {}
            t1 = t2 = None
            for gn in ("f", "i", "g", "o"):
                tag, ro = gate_loc[gn]
                g_sb = gates.tile([BS, H], f32, tag="gate", bufs=4)
                if gn == "g":
                    nc.scalar.activation(out=g_sb, in_=pzt[tag][ro:ro + 64, :],
                                         func=AF.Tanh)
                else:
                    # sigmoid via tanh: keeps ACT on the exp_and_others table
                    nc.scalar.activation(out=g_sb, in_=pzt[tag][ro:ro + 64, :],
                                         func=AF.Tanh, scale=0.5)
                    nc.vector.tensor_scalar(out=g_sb, in0=g_sb,
                                            scalar1=0.5, scalar2=0.5,
                                            op0=mybir.AluOpType.mult,
                                            op1=mybir.AluOpType.add)
                sig[gn] = g_sb
                if gn == "f":
                    t1 = gates.tile([BS, H], f32, tag="tmp", bufs=2)
                    nc.vector.tensor_mul(t1, sig["f"], c_st[0])
                elif gn == "g":
                    t2 = gates.tile([BS, H], f32, tag="tmp", bufs=2)
                    nc.vector.tensor_mul(t2, sig["i"], sig["g"])
                    c_st[0] = state.tile([BS, H], f32, tag="c", name=f"c_{s}")
                    nc.vector.tensor_add(c_st[0], t1, t2)
            tc_sb = gates.tile([BS, H], f32, tag="tmp", bufs=2)
            nc.scalar.activation(out=tc_sb, in_=c_st[0], func=AF.Tanh)
            hbf[0] = small.tile([BS, H], bf16, tag="h_bf", bufs=2,
                                name=f"h_bf_{s}")
            nc.vector.tensor_mul(hbf[0], sig["o"], tc_sb)

        for s in range(S):
            if s > 0:
                emit_hT_transpose(s)
            qT = emit_qT(s)
            if s > 0:
                emit_probs(s - 1)
            pzt = {"FI": pz.tile([128, 512], f32, tag="FI", name=f"pzFI_{s}"),
                   "GO": pz.tile([128, 512], f32, tag="GO", name=f"pzGO_{s}")}
            emit_z_early(s, pzt)
            pe_ = [None] * NG
            ctx_sb = [small.tile([GB, C], bf16, tag=f"ctx_sb{g}", bufs=2,
                                 name=f"ctx{g}_{s}") for g in range(NG)]
            pxT = [None] * NG
            xTc = small.tile([128, 4, BS], bf16, tag="xTc", bufs=2,
                             name=f"xTc_{s}")
            aTs = [None] * NG
            for g in range(NG):
                pe_[g] = pep.tile([128, 512], f32, tag="pe", name=f"pe{g}_{s}")
                emit_att_tanh(s, g, qT, pe_[g])
                aTs[g] = emit_exp_scatter(s, g, pe_[g])
            for g in range(NG):
                emit_post(s, g, ctx_sb[g], aTs[g])
                pxT[g] = ptp.tile([128, 256], bf16, tag="tp", name=f"pxT{g}_{s}")
                emit_ctxT(s, g, ctx_sb[g], pxT, xTc)
            emit_z_late(s, pzt, xTc)
            emit_gates(s, pzt)
        emit_hT_transpose(S)
        emit_probs(S - 1)

    nc.finalize()
    return nc


def _prep_core(inputs, i):
    bsl = slice(i * BS, (i + 1) * BS)
    bh_i = np.asarray(inputs["batch_H"][bsl], np.float32)          # [64, 64, 512]
    text_i = np.asarray(inputs["text"][bsl])                       # [64, 26]
    bh_g = bh_i.reshape(NG, GB, T, C)
    m = {}
    m["bHT"] = np.ascontiguousarray(bh_g.transpose(0, 3, 2, 1)).astype(BF)
    m["bHc"] = np.ascontiguousarray(bh_g.reshape(NG, GB // 2, 128, C)).astype(BF)
    m["wi"] = np.asarray(inputs["Wi"], np.float32).astype(BF)
    m["wh"] = np.asarray(inputs["Wh"], np.float32).astype(BF)
    m["bh"] = np.ascontiguousarray(
        np.asarray(inputs["bh"], np.float32).reshape(4, 128).T)
    wsr = np.ascontiguousarray(
        np.asarray(inputs["Ws"], np.float32)[:, 0].reshape(4, 128).T).astype(BF)
    m["ws"] = np.repeat(wsr[:, :, None], 32, axis=2)
    lk = np.asarray(inputs["lstm_kernel"], np.float32)
    lb = np.asarray(inputs["lstm_bias"], np.float32)
    m["kc"] = lk[:C].astype(BF)
    m["ko"] = (lk[C:] + lb[None, :]).astype(BF)
    m["rr"] = np.asarray(inputs["lstm_rec"], np.float32).astype(BF)
    m["oh"] = (np.arange(NCC)[:, None, None] == text_i.T[None, :, :]).astype(BF)
    m["wg"] = np.asarray(inputs["Wgen"], np.float32).astype(BF)
    m["bg"] = np.tile(np.asarray(inputs["bgen"], np.float32)[None, :], (BS, 1))
    m["ident"] = np.eye(128, dtype=np.float32).astype(BF)
    return m


def kernel(_trace=False, **inputs):
    from concourse import bass_utils
    if "nc" not in _CACHE:
        _CACHE["nc"] = build_bass()
    nc = _CACHE["nc"]
    in_maps = [_prep_core(inputs, i) for i in range(NCORES)]
    res = bass_utils.run_bass_kernel_spmd(nc, in_maps, list(range(NCORES)),
                                          trace=_trace)
    _CACHE["last_result"] = res
    out = np.concatenate([r["out"] for r in res.results], axis=0)
    return out.astype(np.float32)


# revision 12
# speedup vs baseline: 1.6149x; 1.0324x over previous
"""Bass/Trainium2 kernel for attention-LSTM decoder (nn_Attention_49289044688898).

Data-parallel over batch: 512 rows -> 8 NeuronCores x 64 rows. Weights replicated.
Within a core, 64 rows = 2 groups of 32 for the attention; LSTM/q/probs joint.

v2 schedule (vs v1): no DMA transposes (PE transposes + direct-transposed qT
matmul), oh/h@R/probs matmuls hoisted into the tanh window, exp before the
e-scatter DRAM roundtrip with softmax normalization folded into the ctx
PSUM->SBUF copy, AF.Sigmoid for gates.

Per step s (26 steps):
  hT  = transpose(h)                        (PE, 4 transpose-mm)
  qT  = WhT-chunks @ hT                     (PE, 16 mm N=64, k-accum)
  probs(s-1) = hT-mm @ Wgen + bg            (PE + DVE, during tanh window)
  z-partial: onehot@Ko' + h@R               (PE, during tanh window)
  per group g: th = tanh(HprojT + qT)       (DVE add + ACT tanh, 4 chunks)
               e  = ws-quadrant mms         (PE)
               ex = exp(e) (PSUM->SBUF)     (ACT, no max-sub)
               scatter/gather via DRAM -> apad [b, t]
               sum+recip                    (DVE)
               alphaT = PE-transpose(apad); ablk scatter (2 DMA)
               ctx = ablk@bHc mms; scale by 1/sum in PSUM->SBUF copy
  xTc = PE-transpose(ctx)
  z  += xTc @ Kc                            (PE)
  gates: sigmoid/tanh (ACT) + c/h (DVE)
Layouts:
  attention world: [128 part = h_lo, 4 h_hi, 64 t, 32 b]
  context world:   [128 part = (b%2)*64 + t, 16 kt=b//2, 512 c]
  LSTM world:      [64 part = b, free]
"""

import numpy as np
import ml_dtypes
from contextlib import ExitStack

B, T, C, H, NCC, S = 512, 64, 512, 512, 96, 26
NCORES = 8
BS = B // NCORES          # 64 batch rows per core
NG = 2                    # groups per core
GB = BS // NG             # 32 rows per group
BF = ml_dtypes.bfloat16

_CACHE = {}


def build_bass():
    import concourse.bass as bass
    import concourse.bacc as bacc
    import concourse.tile as tile
    import concourse.mybir as mybir

    f32 = mybir.dt.float32
    bf16 = mybir.dt.bfloat16
    AF = mybir.ActivationFunctionType
    AX = mybir.AxisListType

    nc = bacc.Bacc("TRN2", target_bir_lowering=False)

    # ---- DRAM I/O ----
    bHT_d = nc.dram_tensor("bHT", [NG, C, T, GB], bf16, kind="ExternalInput")
    bHc_d = nc.dram_tensor("bHc", [NG, GB // 2, 128, C], bf16, kind="ExternalInput")
    wi_d = nc.dram_tensor("wi", [C, H], bf16, kind="ExternalInput")
    wh_d = nc.dram_tensor("wh", [H, H], bf16, kind="ExternalInput")
    bh_d = nc.dram_tensor("bh", [128, 4], f32, kind="ExternalInput")
    ws_d = nc.dram_tensor("ws", [128, 4, 32], bf16, kind="ExternalInput")
    kc_d = nc.dram_tensor("kc", [C, 4 * H], bf16, kind="ExternalInput")
    rr_d = nc.dram_tensor("rr", [H, 4 * H], bf16, kind="ExternalInput")
    ko_d = nc.dram_tensor("ko", [NCC, 4 * H], bf16, kind="ExternalInput")
    oh_d = nc.dram_tensor("oh", [NCC, S, BS], bf16, kind="ExternalInput")
    wg_d = nc.dram_tensor("wg", [H, NCC], bf16, kind="ExternalInput")
    bg_d = nc.dram_tensor("bg", [BS, NCC], f32, kind="ExternalInput")
    id_d = nc.dram_tensor("ident", [128, 128], bf16, kind="ExternalInput")
    out_d = nc.dram_tensor("out", [BS, S, NCC], f32, kind="ExternalOutput")

    NCH = T * GB // 512  # 4 (t,b)-chunks of 512 per group

    with tile.TileContext(nc) as tc, ExitStack() as ctx:
        big = ctx.enter_context(tc.tile_pool(name="big", bufs=1))
        wpool = ctx.enter_context(tc.tile_pool(name="wpool", bufs=1))
        small = ctx.enter_context(tc.tile_pool(name="small", bufs=2))
        tiny = ctx.enter_context(tc.tile_pool(name="tiny", bufs=4))
        gates = ctx.enter_context(tc.tile_pool(name="gates", bufs=4))
        state = ctx.enter_context(tc.tile_pool(name="state", bufs=2))
        # PSUM pools (8 banks total):
        #   pz:  FI + GO gate accumulators  [128,512] x2     = 2 banks
        #   pep: e quadrant accumulator     [128,512] bufs=2 = 2 banks
        #   pcp: ctx accumulator            [32,512]  bufs=1 = 1 bank
        #   ptp: bf16 PE-transpose outs     [128,256] bufs=1 = 1 bank
        #   psm: qT/probs/sums f32 mm outs  [128,256] bufs=2 = 2 banks
        pz = ctx.enter_context(tc.tile_pool(name="pz", bufs=1, space="PSUM"))
        pep = ctx.enter_context(tc.tile_pool(name="pep", bufs=2, space="PSUM"))
        pcp = ctx.enter_context(tc.tile_pool(name="pcp", bufs=1, space="PSUM"))
        ptp = ctx.enter_context(tc.tile_pool(name="ptp", bufs=1, space="PSUM"))
        psm = ctx.enter_context(tc.tile_pool(name="psm", bufs=2, space="PSUM"))

        dma = nc.sync
        import concourse.bass as _b

        # ---- load weights / big tensors ----
        bHc = [big.tile([128, GB // 2, C], bf16, tag=f"bHc{g}", name=f"bHc{g}")
               for g in range(NG)]
        for g in range(NG):
            dma.dma_start(out=bHc[g], in_=bHc_d[g].rearrange("k p c -> p k c"))
        # batch_H^T (prolog only; shares slots with tanh buffers)
        bHT = [big.tile([128, 4, T * GB], bf16, tag=f"th{g}", name=f"bHT{g}")
               for g in range(NG)]
        for g in range(NG):
            dma.dma_start(
                out=bHT[g],
                in_=bHT_d[g].rearrange("(ch cl) t b -> cl ch (t b)", cl=128))

        wi = wpool.tile([128, 4, H], bf16, tag="wi")
        dma.dma_start(out=wi, in_=wi_d[:].rearrange("(ch cl) h -> cl ch h", cl=128))
        wh = wpool.tile([128, 4, H], bf16, tag="wh")
        dma.dma_start(out=wh, in_=wh_d[:].rearrange("(hh hl) h -> hl hh h", hl=128))
        bh = wpool.tile([128, 4], f32, tag="bh")
        dma.dma_start(out=bh, in_=bh_d[:])
        ws = wpool.tile([128, 4, 32], bf16, tag="ws")
        dma.dma_start(out=ws, in_=ws_d[:])
        kc = wpool.tile([128, 4, 4 * H], bf16, tag="kc")
        dma.dma_start(out=kc, in_=kc_d[:].rearrange("(kh kl) n -> kl kh n", kl=128))
        rr = wpool.tile([128, 4, 4 * H], bf16, tag="rr")
        dma.dma_start(out=rr, in_=rr_d[:].rearrange("(kh kl) n -> kl kh n", kl=128))
        ko = wpool.tile([NCC, 4 * H], bf16, tag="ko")
        dma.dma_start(out=ko, in_=ko_d[:])
        oh = wpool.tile([NCC, S, BS], bf16, tag="oh")
        dma.dma_start(out=oh, in_=oh_d[:])
        wg = wpool.tile([128, 4, NCC], bf16, tag="wg")
        dma.dma_start(out=wg, in_=wg_d[:].rearrange("(hh hl) n -> hl hh n", hl=128))
        bg = wpool.tile([BS, NCC], f32, tag="bg")
        dma.dma_start(out=bg, in_=bg_d[:])
        ident = wpool.tile([128, 128], bf16, tag="ident")
        dma.dma_start(out=ident, in_=id_d[:])
        ones = wpool.tile([T, 1], bf16, tag="ones")
        nc.vector.memset(ones, 1.0)

        # block-diag alpha holders (zeroed once)
        ablk = [wpool.tile([128, GB // 2, GB], bf16, tag=f"ablk{g}", name=f"ablk{g}")
                for g in range(NG)]
        for g in range(NG):
            nc.vector.memset(ablk[g], 0.0)

        # initial state
        hT = [state.tile([128, 4, BS], bf16, tag="hT", name="hT0")]
        nc.vector.memset(hT[0], 0.0)
        c_st = [state.tile([BS, H], f32, tag="c", name="c0")]
        nc.vector.memset(c_st[0], 0.0)
        hbf = [None]

        # ---- prolog: HprojT[g] = (batch_H @ Wi)^T + bh ----
        hprojT = [big.tile([128, 4, T * GB], bf16, tag=f"hp{g}", name=f"hp{g}")
                  for g in range(NG)]
        for g in range(NG):
            for m in range(4):
                for n in range(NCH):
                    ps = pz.tile([128, 512], f32, tag="FI" if g == 0 else "GO")
                    for k in range(4):
                        nc.tensor.matmul(
                            ps,
                            wi[:, k, m * 128:(m + 1) * 128],
                            bHT[g][:, k, n * 512:(n + 1) * 512],
                            start=(k == 0), stop=(k == 3),
                        )
                    nc.scalar.activation(
                        out=hprojT[g][:, m, n * 512:(n + 1) * 512], in_=ps,
                        func=AF.Identity, bias=bh[:, m:m + 1], scale=1.0,
                    )

        def bcast_t(ap2):
            # [128, GB(b)] -> [128, T(t, stride0), GB(b)]
            return _b.AP(tensor=ap2.tensor, offset=ap2.offset,
                         ap=[ap2.ap[0], [0, T], ap2.ap[1]])

        gate_sl = {"f": 1, "i": 0, "g": 2, "o": 3}
        # gate -> (psum tag, row offset): f/i share FI bank, g/o share GO bank
        gate_loc = {"f": ("FI", 0), "i": ("FI", 64), "g": ("GO", 0), "o": ("GO", 64)}

        def emit_hT_transpose(s):
            # h_bf [64, 512] -> hT [128, 4, 64] via 4 PE transposes
            phT = ptp.tile([128, 256], bf16, tag="tp", name=f"phT_{s}")
            for m in range(4):
                nc.tensor.transpose(phT[:, m * 64:(m + 1) * 64],
                                    hbf[0][:, m * 128:(m + 1) * 128],
                                    ident[0:BS, 0:BS])
            hT[0] = state.tile([128, 4, BS], bf16, tag="hT", name=f"hT_{s}")
            nc.vector.tensor_copy(hT[0], phT)

        def emit_qT(s):
            # qT[h',b] = sum_h Wh[h,h'] hT[h,b]; m-outer so chunk m is
            # copied out as soon as its k-accumulation finishes.
            pqT = psm.tile([128, 256], f32, tag="pq", name=f"pqT_{s}")
            qT = small.tile([128, 4, BS], bf16, tag="qT", bufs=2, name=f"qT_{s}")
            for m in range(4):
                for k in range(4):
                    nc.tensor.matmul(pqT[:, m * 64:(m + 1) * 64],
                                     wh[:, k, m * 128:(m + 1) * 128],
                                     hT[0][:, k, :],
                                     start=(k == 0), stop=(k == 3))
                nc.vector.tensor_copy(qT[:, m, :], pqT[:, m * 64:(m + 1) * 64])
            return qT

        def emit_probs(sm1):
            # probs(sm1) = h(sm1) @ Wgen + bg, from hT
            pp = psm.tile([128, 256], f32, tag="pq", name=f"pp_{sm1}")
            for k in range(4):
                nc.tensor.matmul(pp[0:BS, 0:NCC], hT[0][:, k, :], wg[:, k, :],
                                 start=(k == 0), stop=(k == 3))
            pr = small.tile([BS, NCC], f32, tag="pr", bufs=2, name=f"pr_{sm1}")
            nc.vector.tensor_add(pr, pp[0:BS, 0:NCC], bg)
            dma.dma_start(out=out_d[:, sm1, :], in_=pr)

        def emit_z_early(s, pzt):
            # onehot@Ko' (start) + h@R during the tanh window
            for gn in "figo":
                tag, ro = gate_loc[gn]
                zsl = slice(gate_sl[gn] * 512, (gate_sl[gn] + 1) * 512)
                nc.tensor.matmul(pzt[tag][ro:ro + 64, :], oh[:, s, :],
                                 ko[:, zsl], start=True, stop=False,
                                 tile_position=(0, ro))
            for k in range(4):
                for gn in "figo":
                    tag, ro = gate_loc[gn]
                    zsl = slice(gate_sl[gn] * 512, (gate_sl[gn] + 1) * 512)
                    nc.tensor.matmul(pzt[tag][ro:ro + 64, :], hT[0][:, k, :],
                                     rr[:, k, zsl], start=False, stop=False,
                                     tile_position=(0, ro))

        def emit_att_tanh(s, g, qT, pe_):
            # DVE add + ACT tanh + e quadrant mms for group g
            gsl_b = slice(g * GB, (g + 1) * GB)
            th = big.tile([128, 4, T * GB], bf16, tag=f"th{g}", name=f"th{g}_{s}")
            for k in range(4):
                nc.vector.tensor_add(
                    th[:, k, :].rearrange("p (t b) -> p t b", t=T),
                    hprojT[g][:, k, :].rearrange("p (t b) -> p t b", t=T),
                    bcast_t(qT[:, k, gsl_b]))
                nc.scalar.activation(out=th[:, k, :], in_=th[:, k, :], func=AF.Tanh)
                for j in range(NCH):
                    bp = 32 * j
                    nc.tensor.matmul(pe_[bp:bp + 32, :], ws[:, k, :],
                                     th[:, k, j * 512:(j + 1) * 512],
                                     start=(k == 0), stop=(k == 3),
                                     tile_position=(0, bp))

        def emit_exp_scatter(s, g, pe_):
            # exp on the PSUM layout: est[32j, tl*32+b] = ex(t=16j+tl, b).
            # ablk (block-diag) is written DIRECTLY from est (2 DMAs, one
            # per b-parity); alphaT (only feeds the denominator matmul)
            # via one merged DMA on the gpsimd queue.
            est = small.tile([128, 512], bf16, tag=f"est{g}", bufs=1,
                             name=f"est{g}_{s}")
            nc.scalar.activation(out=est, in_=pe_, func=AF.Exp)
            ea = est[:]
            pp = ea.ap[0][0]
            alphaT = small.tile([T, GB], bf16, tag=f"alphaT{g}", bufs=2,
                                name=f"alphaT{g}_{s}")
            at = alphaT[:]
            for j in range(4):
                esl = est[32 * j:32 * j + 1, :]
                srcj = _b.AP(tensor=esl.tensor, offset=esl.offset,
                             ap=[[esl.ap[0][0], 1], [GB, T // 4], [1, GB]])
                nc.gpsimd.dma_start(out=alphaT[16 * j:16 * (j + 1), :], in_=srcj)
            # ablk block-diag scatter (2 DMAs, sync queue)
            ab = ablk[g][:]
            for par in (0, 1):
                srcp = _b.AP(tensor=at.tensor, offset=at.offset + par * at.ap[1][0],
                             ap=[[at.ap[0][0], T], [2 * at.ap[1][0], GB // 2]])
                dst = _b.AP(tensor=ab.tensor,
                            offset=ab.offset + par * (64 * ab.ap[0][0] + ab.ap[2][0]),
                            ap=[[ab.ap[0][0], T], [ab.ap[1][0] + 2 * ab.ap[2][0], GB // 2]])
                dma.dma_start(out=dst, in_=srcp)
            return alphaT

        def emit_post(s, g, ctx_sb, alphaT):
            # ctx_sb: per-group [GB, C] tile (base partition 0)
            # denominator: sums[b] = alphaT^T @ ones  (one matmul, N=1)
            psums = psm.tile([128, 256], f32, tag="pq", name=f"psm{g}_{s}")
            nc.tensor.matmul(psums[0:GB, 0:1], alphaT, ones,
                             start=True, stop=True)
            rcp = tiny.tile([GB, 1], f32, tag=f"rcp{g}")
            nc.vector.reciprocal(rcp, psums[0:GB, 0:1])
            pctx = pcp.tile([GB, C], f32, tag="ctx")
            for kt in range(GB // 2):
                nc.tensor.matmul(pctx, ablk[g][:, kt, :], bHc[g][:, kt, :],
                                 start=(kt == 0), stop=(kt == GB // 2 - 1))
            nc.vector.tensor_scalar_mul(ctx_sb, pctx, rcp)

        def emit_ctxT(s, g, ctx_sb, pxT, xTc):
            # ctx rows of group g -> xTc[:, k, g*32:(g+1)*32]
            for k in range(4):
                nc.tensor.transpose(pxT[g][:, k * GB:(k + 1) * GB],
                                    ctx_sb[:, k * 128:(k + 1) * 128],
                                    ident[0:GB, 0:GB])
            src = pxT[g][:, 0:128].rearrange("p (k b) -> p k b", k=4)
            nc.vector.tensor_copy(xTc[:, :, g * GB:(g + 1) * GB], src)

        def emit_z_late(s, pzt, xTc):
            for pair in ("fi", "go"):
                for k in range(4):
                    for gn in pair:
                        tag, ro = gate_loc[gn]
                        zsl = slice(gate_sl[gn] * 512, (gate_sl[gn] + 1) * 512)
                        nc.tensor.matmul(pzt[tag][ro:ro + 64, :], xTc[:, k, :],
                                         kc[:, k, zsl], start=False,
                                         stop=(k == 3), tile_position=(0, ro))

        def emit_gates(s, pzt):
            sig = {}
            t1 = t2 = None
            for gn in ("f", "i", "g", "o"):
                tag, ro = gate_loc[gn]
                g_sb = gates.tile([BS, H], f32, tag="gate", bufs=4)
                if gn == "g":
                    nc.scalar.activation(out=g_sb, in_=pzt[tag][ro:ro + 64, :],
                                         func=AF.Tanh)
                else:
                    # sigmoid via tanh: keeps ACT on the exp_and_others table
                    nc.scalar.activation(out=g_sb, in_=pzt[tag][ro:ro + 64, :],
                                         func=AF.Tanh, scale=0.5)
                    nc.vector.tensor_scalar(out=g_sb, in0=g_sb,
                                            scalar1=0.5, scalar2=0.5,
                                            op0=mybir.AluOpType.mult,
                                            op1=mybir.AluOpType.add)
                sig[gn] = g_sb
                if gn == "f":
                    t1 = gates.tile([BS, H], f32, tag="tmp", bufs=2)
                    nc.vector.tensor_mul(t1, sig["f"], c_st[0])
                elif gn == "g":
                    t2 = gates.tile([BS, H], f32, tag="tmp", bufs=2)
                    nc.vector.tensor_mul(t2, sig["i"], sig["g"])
                    c_st[0] = state.tile([BS, H], f32, tag="c", name=f"c_{s}")
                    nc.vector.tensor_add(c_st[0], t1, t2)
            tc_sb = gates.tile([BS, H], f32, tag="tmp", bufs=2)
            nc.scalar.activation(out=tc_sb, in_=c_st[0], func=AF.Tanh)
            hbf[0] = small.tile([BS, H], bf16, tag="h_bf", bufs=2,
                                name=f"h_bf_{s}")
            nc.vector.tensor_mul(hbf[0], sig["o"], tc_sb)

        for s in range(S):
            if s > 0:
                emit_hT_transpose(s)
            qT = emit_qT(s)
            if s > 0:
                emit_probs(s - 1)
            pzt = {"FI": pz.tile([128, 512], f32, tag="FI", name=f"pzFI_{s}"),
                   "GO": pz.tile([128, 512], f32, tag="GO", name=f"pzGO_{s}")}
            emit_z_early(s, pzt)
            pe_ = [None] * NG
            ctx_sb = [small.tile([GB, C], bf16, tag=f"ctx_sb{g}", bufs=2,
                                 name=f"ctx{g}_{s}") for g in range(NG)]
            pxT = [None] * NG
            xTc = small.tile([128, 4, BS], bf16, tag="xTc", bufs=2,
                             name=f"xTc_{s}")
            aTs = [None] * NG
            for g in range(NG):
                pe_[g] = pep.tile([128, 512], f32, tag="pe", name=f"pe{g}_{s}")
                emit_att_tanh(s, g, qT, pe_[g])
                aTs[g] = emit_exp_scatter(s, g, pe_[g])
            for g in range(NG):
                emit_post(s, g, ctx_sb[g], aTs[g])
                pxT[g] = ptp.tile([128, 256], bf16, tag="tp", name=f"pxT{g}_{s}")
                emit_ctxT(s, g, ctx_sb[g], pxT, xTc)
            emit_z_late(s, pzt, xTc)
            emit_gates(s, pzt)
        emit_hT_transpose(S)
        emit_probs(S - 1)

    nc.finalize()
    return nc


def _prep_core(inputs, i):
    bsl = slice(i * BS, (i + 1) * BS)
    bh_i = np.asarray(inputs["batch_H"][bsl], np.float32)          # [64, 64, 512]
    text_i = np.asarray(inputs["text"][bsl])                       # [64, 26]
    bh_g = bh_i.reshape(NG, GB, T, C)
    m = {}
    m["bHT"] = np.ascontiguousarray(bh_g.transpose(0, 3, 2, 1)).astype(BF)
    m["bHc"] = np.ascontiguousarray(bh_g.reshape(NG, GB // 2, 128, C)).astype(BF)
    m["wi"] = np.asarray(inputs["Wi"], np.float32).astype(BF)
    m["wh"] = np.asarray(inputs["Wh"], np.float32).astype(BF)
    m["bh"] = np.ascontiguousarray(
        np.asarray(inputs["bh"], np.float32).reshape(4, 128).T)
    wsr = np.ascontiguousarray(
        np.asarray(inputs["Ws"], np.float32)[:, 0].reshape(4, 128).T).astype(BF)
    m["ws"] = np.repeat(wsr[:, :, None], 32, axis=2)
    lk = np.asarray(inputs["lstm_kernel"], np.float32)
    lb = np.asarray(inputs["lstm_bias"], np.float32)
    m["kc"] = lk[:C].astype(BF)
    m["ko"] = (lk[C:] + lb[None, :]).astype(BF)
    m["rr"] = np.asarray(inputs["lstm_rec"], np.float32).astype(BF)
    m["oh"] = (np.arange(NCC)[:, None, None] == text_i.T[None, :, :]).astype(BF)
    m["wg"] = np.asarray(inputs["Wgen"], np.float32).astype(BF)
    m["bg"] = np.tile(np.asarray(inputs["bgen"], np.float32)[None, :], (BS, 1))
    m["ident"] = np.eye(128, dtype=np.float32).astype(BF)
    return m


def kernel(_trace=False, **inputs):
    from concourse import bass_utils
    if "nc" not in _CACHE:
        _CACHE["nc"] = build_bass()
    nc = _CACHE["nc"]
    in_maps = [_prep_core(inputs, i) for i in range(NCORES)]
    res = bass_utils.run_bass_kernel_spmd(nc, in_maps, list(range(NCORES)),
                                          trace=_trace)
    _CACHE["last_result"] = res
    out = np.concatenate([r["out"] for r in res.results], axis=0)
    return out.astype(np.float32)
